# revision 25
# baseline (speedup 1.0000x reference)
"""Barrel shifter right 64 (zero-fill), batch 2097152, on 8 NeuronCores.

Layout: row-major. Each SBUF work tile holds 4096 rows: partition p carries 32
consecutive rows (spans), each span padded to 96 bf16 elements (32 zero guard +
64 data). A stage's shift-by-sa is a free-dim offset view whose low lanes read
the zero guard.

Engine split — DVE (the only engine with copy_predicated, which has no fast
perf mode) runs NOTHING but the six predicated mux copies:
  - DVE: per stage one 1x copy_predicated on int32 BF16 PAIRS (the mux select
    is per row, so adjacent lanes share it; every shift >= 2 is pair-aligned),
    halving the 1x element count. Stages 1..5 predicate directly on the raw
    f32 select bit broadcast across the span, bitcast to int32 (nonzero test;
    1.0f != 0) — no mask materialization. They run in place with REVERSED
    innermost APs: processing high->low guarantees each shifted read (at k-w)
    precedes that position's overwrite, for any w.
  - ScalarE: f32->bf16 in-copy, stage-0's inverted mask (select bit minus 1:
    nonzero exactly when the bit is clear — activation Copy with bias -1
    writing int32), and stage-0's shift-by-1 copy A->B (odd bf16 offset
    breaks the int32 pairing, hence the copy + inverted-predicate form).
  - GPSIMD: bf16->f32 out-copy.

All tile classes rotate through 3 slots and the output DMA lags 3 tiles, so
the sync engine issues input DMAs back-to-back and each tile's ~7us HBM load
is in flight ~2 tiles before its consumption.
"""

import sys

if "/opt/trn_rl_repo" not in sys.path:
    sys.path.insert(0, "/opt/trn_rl_repo")

import numpy as np

B_TOTAL = 2097152
NBITS = 64
NCTRL = 6
NCORES = 8
R_FULL = B_TOTAL // NCORES  # 262144 rows per core

P = 128
SPANS = 32                  # rows per partition per tile
TILE_ROWS = P * SPANS       # 4096
PITCH = 96                  # bf16 elems: guard(32) + bits(64)
GUARD = 32
W32 = NBITS // 2            # 32 int32 lanes per span
PITCH32 = PITCH // 2        # 48
GUARD32 = GUARD // 2        # 16
FD = SPANS * NBITS          # 2048
SFD = SPANS * NCTRL         # 192
NS = 3                      # slot count for every rotating tile class

_built = {}


def build(rows, sim_sync=False):
    # sim_sync inserts drains between same-engine dependent ops purely to
    # satisfy CoreSim's conservative OOO-engine race model; on hardware the
    # ops are all far above the ~266ns write-flush threshold (the proven
    # baseline relies on the same adjacency).
    import concourse.bass as bass
    from concourse import mybir

    f32 = mybir.dt.float32
    bf16 = mybir.dt.bfloat16
    i32 = mybir.dt.int32
    Alu = mybir.AluOpType
    Act = mybir.ActivationFunctionType

    nt = rows // TILE_ROWS
    assert rows % TILE_ROWS == 0
    assert nt >= 2 * NS

    nc = bass.Bass()
    data = nc.declare_dram_parameter("data", [rows, NBITS], f32, isOutput=False)
    shift = nc.declare_dram_parameter("shift", [rows, NCTRL], f32, isOutput=False)
    out = nc.declare_dram_parameter("out", [rows, NBITS], f32, isOutput=True)

    data_r = data.rearrange("(n p t) k -> n p (t k)", p=P, t=SPANS)
    shift_r = shift.rearrange("(n p t) k -> n p (t k)", p=P, t=SPANS)
    out_r = out.rearrange("(n p t) k -> n p (t k)", p=P, t=SPANS)

    def sb(name, shape, dt):
        return nc.alloc_sbuf_tensor(name, shape, dt)

    dtile = [sb(f"dtile{j}", [P, FD], f32) for j in range(NS)]
    stile = [sb(f"stile{j}", [P, SFD], f32) for j in range(NS)]
    # stage-0 masks share the 48-pitch span structure of the work tiles so
    # their APs lower with the same dimension structure as copy_predicated's
    msk = [sb(f"msk{j}", [P, SPANS * PITCH32], i32) for j in range(NS)]
    wkA = [sb(f"wkA{j}", [P, SPANS * PITCH], bf16) for j in range(NS)]
    wkB = [sb(f"wkB{j}", [P, SPANS * PITCH], bf16) for j in range(NS)]
    otile = [sb(f"otile{j}", [P, FD], f32) for j in range(NS)]

    def spans(t, off=GUARD):
        # [P, SPANS, NBITS] bf16 view at span-local offset `off`
        return t.ap().rearrange("p (t c) -> p t c", c=PITCH)[:, :, off:off + NBITS]

    def spans32(t, off=GUARD32, rev=False):
        # [P, SPANS, W32] int32 (bf16-pair) view at span-local int32 offset
        v = t.ap().bitcast(i32).rearrange("p (t c) -> p t c", c=PITCH32)[
            :, :, off:off + W32
        ]
        return v[:, :, ::-1] if rev else v

    with (
        nc.Block() as block,
        nc.semaphore("s_din0") as s_din0,
        nc.semaphore("s_din1") as s_din1,
        nc.semaphore("s_din2") as s_din2,
        nc.semaphore("s_dout0") as s_dout0,
        nc.semaphore("s_dout1") as s_dout1,
        nc.semaphore("s_dout2") as s_dout2,
        nc.semaphore("s_pre") as s_pre,
        nc.semaphore("s_vec") as s_vec,
        nc.semaphore("s_oc") as s_oc,
        nc.semaphore("s_zero") as s_zero,
    ):
        s_din = [s_din0, s_din1, s_din2]
        s_dout = [s_dout0, s_dout1, s_dout2]

        @block.sync
        def _(sp):
            for n in range(nt):
                r = n % NS
                if n >= NS:
                    # input slot r free once ScalarE's in-copy (dtile) and
                    # DVE (stile, read by the predicates) of tile n-NS done
                    sp.wait_ge(s_pre, n - NS + 1)
                    sp.wait_ge(s_vec, n - NS + 1)
                sp.dma_start(
                    out=dtile[r].ap(), in_=data_r[n]
                ).then_inc(s_din[r], 16)
                sp.dma_start(
                    out=stile[r].ap(), in_=shift_r[n]
                ).then_inc(s_din[r], 16)
                if n >= NS:
                    m = n - NS
                    sp.wait_ge(s_oc, m + 1)  # out-copy of tile m done
                    sp.dma_start(
                        out=out_r[m], in_=otile[m % NS].ap()
                    ).then_inc(s_dout[m % NS], 16)
            for m in range(nt - NS, nt):
                sp.wait_ge(s_oc, m + 1)
                sp.dma_start(
                    out=out_r[m], in_=otile[m % NS].ap()
                ).then_inc(s_dout[m % NS], 16)
            for j in range(NS):
                sp.wait_ge(s_dout[j], 16 * len(range(j, nt, NS)))

        @block.scalar
        def _(s):
            s.wait_ge(s_zero, 1)
            for n in range(nt):
                r = n % NS
                if n >= NS:
                    # work pair r free once DVE(n-NS) is done with it, and
                    # its B read out by the out-copy of tile n-NS
                    s.wait_ge(s_vec, n - NS + 1)
                    s.wait_ge(s_oc, n - NS + 1)
                s.wait_ge(s_din[r], 32 * (n // NS + 1))
                d3 = dtile[r].ap().rearrange("p (t k) -> p t k", k=NBITS)
                A, Bw = wkA[r], wkB[r]
                s.copy(spans(A), d3)
                # stage-0 inverted mask: bit - 1 (nonzero iff bit clear);
                # also spaces the dependent in-copy -> stage-0-copy pair
                st3 = stile[r].ap().rearrange("p (t j) -> p t j", j=NCTRL)
                m3 = msk[r].ap().rearrange("p (t k) -> p t k", k=PITCH32)[
                    :, :, 0:W32
                ]
                if sim_sync:
                    s.drain()
                s.activation(
                    m3,
                    st3[:, :, 5:6].broadcast_to([P, SPANS, W32]),
                    Act.Copy,
                    bias=-1.0,
                )
                if sim_sync:
                    s.drain()
                # stage 0 shifted copy: B = A >> 1 lane (src reads one guard
                # zero at the span head)
                s.copy(spans(Bw), spans(A, GUARD - 1)).then_inc(s_pre, 1)

        @block.gpsimd
        def _(g):
            g.wait_ge(s_zero, 1)
            for m in range(nt):
                o = m % NS
                g.wait_ge(s_vec, m + 1)       # B_m holds tile m's result
                if m >= NS:
                    # otile slot o drained for tile m-NS
                    g.wait_ge(s_dout[o], 16 * (m // NS))
                if sim_sync:
                    g.drain()
                o3 = otile[o].ap().rearrange("p (t k) -> p t k", k=NBITS)
                g.tensor_copy(o3, spans(wkB[o])).then_inc(s_oc, 1)

        @block.vector
        def _(v):
            # zero the work tiles once so every guard lane reads 0
            for j in range(NS):
                v.memset(wkA[j].ap(), 0.0)
            for j in range(NS):
                v.memset(wkB[j].ap(), 0.0)
            if sim_sync:
                v.drain()
            # re-clear a sliver as the inc carrier: an ENGINE op (not a
            # seq-only sem_inc) so the zeroing is provably ordered before it
            v.memset(wkB[0].ap()[:, 0:2], 0.0).then_inc(s_zero, 1)
            for n in range(nt):
                r = n % NS
                v.wait_ge(s_pre, n + 1)
                A, Bw = wkA[r], wkB[r]
                # copy_predicated requires an integer-typed mask; the f32 bit
                # pattern of 1.0 is nonzero, so a bitcast view keeps semantics
                sti3 = stile[r].ap().bitcast(i32).rearrange(
                    "p (t j) -> p t j", j=NCTRL
                )
                m3 = msk[r].ap().rearrange("p (t k) -> p t k", k=PITCH32)[
                    :, :, 0:W32
                ]

                def bitbrd(i):
                    # stage-i select bit (shift[:, 5-i]) broadcast across the
                    # 32 int32 pair-lanes of its span
                    return sti3[:, :, 5 - i:6 - i].broadcast_to(
                        [P, SPANS, W32]
                    )

                # stage 0: B holds shift-by-1(A); predicate the unshifted A
                # over it where the select bit is CLEAR.
                v.copy_predicated(spans32(Bw), m3, spans32(A))
                # stages 1..5 in place on B, reversed inner order
                for i in range(1, 6):
                    w = (1 << i) // 2
                    if sim_sync:
                        v.drain()
                    ins = v.copy_predicated(
                        spans32(Bw, rev=True),
                        bitbrd(i),
                        spans32(Bw, GUARD32 - w, rev=True),
                    )
                    if i == 5:
                        ins.then_inc(s_vec, 1)

    return nc


def _get(rows):
    if rows not in _built:
        _built[rows] = build(rows)
    return _built[rows]


def run_cores(data, shift, rows, trace=False):
    from concourse.bass_utils import run_bass_kernel_spmd

    nc = _get(rows)
    ncores = data.shape[0] // rows
    in_maps = [
        {
            "data": np.ascontiguousarray(data[i * rows:(i + 1) * rows]),
            "shift": np.ascontiguousarray(shift[i * rows:(i + 1) * rows]),
        }
        for i in range(ncores)
    ]
    res = run_bass_kernel_spmd(nc, in_maps, list(range(ncores)), trace=trace)
    full = np.concatenate([res.results[i]["out"] for i in range(ncores)], axis=0)
    return full, res


def kernel(data, shift):
    data = np.ascontiguousarray(np.asarray(data), dtype=np.float32)
    shift = np.ascontiguousarray(np.asarray(shift), dtype=np.float32)
    full, _ = run_cores(data, shift, R_FULL)
    return full.astype(np.float32, copy=False)


# revision 37
# speedup vs baseline: 1.4027x; 1.4027x over previous
"""Barrel shifter right 64 (zero-fill), batch 2097152, on 8 NeuronCores.

Layout: row-major. Each SBUF work tile holds 4096 rows: partition p carries 32
consecutive rows (spans), each span padded to 96 bf16 elements (32 zero guard +
64 data). A stage's shift-by-sa is a free-dim offset view whose low lanes read
the zero guard.

Engine split — DVE (the only engine with copy_predicated, which has no fast
perf mode) runs NOTHING but the six predicated mux copies:
  - DVE: per stage one 1x copy_predicated on int32 BF16 PAIRS (the mux select
    is per row, so adjacent lanes share it; every shift >= 2 is pair-aligned),
    halving the 1x element count. Stages 1..5 predicate directly on the raw
    f32 select bit broadcast across the span, bitcast to int32 (nonzero test;
    1.0f != 0) — no mask materialization. They run in place with REVERSED
    innermost APs: processing high->low guarantees each shifted read (at k-w)
    precedes that position's overwrite, for any w.
  - Stage 0 (sa=1) has an odd bf16 offset that breaks the int32 pairing, so
    it takes the copy + inverted-predicate form: ScalarE writes the shifted
    copy A->B and DVE predicates the unshifted A over it where the select
    bit is CLEAR. The inverted mask (select bit minus 1: nonzero exactly
    when clear) is the one materialized mask; DVE computes it before its
    s_pre wait so it hides inside ScalarE's chain.
  - ScalarE: f32->bf16 in-copy, the bf16->f32 out-copy of tile n-2 (which
    also spaces the dependent in-copy -> stage-0-copy pair), and stage-0's
    shift-by-1 copy. GPSIMD is deliberately idle: its slow software tensor
    ops contend with DVE for SBUF ports (measured: a 9us/tile GPSIMD copy
    doubled every DVE predicated-copy's duration).

All tile classes rotate through 3 slots and the output DMA lags 3 tiles, so
the sync engine issues input DMAs back-to-back and each tile's ~7us HBM load
is in flight ~2 tiles before its consumption.
"""

import sys

if "/opt/trn_rl_repo" not in sys.path:
    sys.path.insert(0, "/opt/trn_rl_repo")

import numpy as np

B_TOTAL = 2097152
NBITS = 64
NCTRL = 6
NCORES = 8
R_FULL = B_TOTAL // NCORES  # 262144 rows per core

P = 128
SPANS = 64                  # rows per partition per tile
TILE_ROWS = P * SPANS       # 8192
PITCH = 96                  # bf16 elems: guard(32) + bits(64)
GUARD = 32
W32 = NBITS // 2            # 32 int32 lanes per span
PITCH32 = PITCH // 2        # 48
GUARD32 = GUARD // 2        # 16
FD = SPANS * NBITS          # 2048
SFD = SPANS * NCTRL         # 192
NS = 3                      # slot count for every rotating tile class

_built = {}


def build(rows, sim_sync=False):
    # sim_sync inserts drains between same-engine dependent ops purely to
    # satisfy CoreSim's conservative OOO-engine race model; on hardware the
    # ops are all far above the ~266ns write-flush threshold (the proven
    # baseline relies on the same adjacency).
    import concourse.bass as bass
    from concourse import mybir

    f32 = mybir.dt.float32
    bf16 = mybir.dt.bfloat16
    i32 = mybir.dt.int32
    Alu = mybir.AluOpType
    Act = mybir.ActivationFunctionType

    nt = rows // TILE_ROWS
    assert rows % TILE_ROWS == 0
    assert nt >= 2 * NS

    nc = bass.Bass()
    data = nc.declare_dram_parameter("data", [rows, NBITS], f32, isOutput=False)
    shift = nc.declare_dram_parameter("shift", [rows, NCTRL], f32, isOutput=False)
    out = nc.declare_dram_parameter("out", [rows, NBITS], f32, isOutput=True)

    data_r = data.rearrange("(n p t) k -> n p (t k)", p=P, t=SPANS)
    shift_r = shift.rearrange("(n p t) k -> n p (t k)", p=P, t=SPANS)
    out_r = out.rearrange("(n p t) k -> n p (t k)", p=P, t=SPANS)

    def sb(name, shape, dt):
        return nc.alloc_sbuf_tensor(name, shape, dt)

    dtile = [sb(f"dtile{j}", [P, FD], f32) for j in range(NS)]
    stile = [sb(f"stile{j}", [P, SFD], f32) for j in range(NS)]
    # The stage-0 mask shares the 48-pitch span structure of the work tiles
    # so its AP lowers with the same dimension structure as copy_predicated's.
    # A single tile suffices: DVE is both its writer (the pre-s_pre
    # tensor_scalar) and its reader (pred0), strictly ordered.
    msk = sb("msk", [P, SPANS * PITCH32], i32)
    wkA = [sb(f"wkA{j}", [P, SPANS * PITCH], bf16) for j in range(NS)]
    wkB = [sb(f"wkB{j}", [P, SPANS * PITCH], bf16) for j in range(NS)]
    otile = [sb(f"otile{j}", [P, FD], f32) for j in range(NS)]

    def spans(t, off=GUARD):
        # [P, SPANS, NBITS] bf16 view at span-local offset `off`
        return t.ap().rearrange("p (t c) -> p t c", c=PITCH)[:, :, off:off + NBITS]

    def spans32(t, off=GUARD32, rev=False):
        # [P, SPANS, W32] int32 (bf16-pair) view at span-local int32 offset
        v = t.ap().bitcast(i32).rearrange("p (t c) -> p t c", c=PITCH32)[
            :, :, off:off + W32
        ]
        return v[:, :, ::-1] if rev else v

    with (
        nc.Block() as block,
        nc.semaphore("s_din0") as s_din0,
        nc.semaphore("s_din1") as s_din1,
        nc.semaphore("s_din2") as s_din2,
        nc.semaphore("s_dout0") as s_dout0,
        nc.semaphore("s_dout1") as s_dout1,
        nc.semaphore("s_dout2") as s_dout2,
        nc.semaphore("s_pre") as s_pre,
        nc.semaphore("s_vec") as s_vec,
        nc.semaphore("s_oc") as s_oc,
        nc.semaphore("s_zero") as s_zero,
    ):
        s_din = [s_din0, s_din1, s_din2]
        s_dout = [s_dout0, s_dout1, s_dout2]

        @block.sync
        def _(sp):
            for n in range(nt):
                r = n % NS
                if n >= NS:
                    # input slot r free once ScalarE's in-copy (dtile) and
                    # DVE (stile, read by the predicates) of tile n-NS done
                    sp.wait_ge(s_pre, n - NS + 1)
                    sp.wait_ge(s_vec, n - NS + 1)
                sp.dma_start(
                    out=dtile[r].ap(), in_=data_r[n]
                ).then_inc(s_din[r], 16)
                sp.dma_start(
                    out=stile[r].ap(), in_=shift_r[n]
                ).then_inc(s_din[r], 16)
                if n >= NS:
                    m = n - NS
                    sp.wait_ge(s_oc, m + 1)  # out-copy of tile m done
                    sp.dma_start(
                        out=out_r[m], in_=otile[m % NS].ap()
                    ).then_inc(s_dout[m % NS], 16)
            for m in range(nt - NS, nt):
                sp.wait_ge(s_oc, m + 1)
                sp.dma_start(
                    out=out_r[m], in_=otile[m % NS].ap()
                ).then_inc(s_dout[m % NS], 16)
            for j in range(NS):
                sp.wait_ge(s_dout[j], 16 * len(range(j, nt, NS)))

        @block.scalar
        def _(s):
            s.wait_ge(s_zero, 1)
            for n in range(nt):
                r = n % NS
                if n >= NS:
                    # work pair r free once DVE(n-NS) is done with it (its B
                    # was read out by this engine's own out-copy, in order)
                    s.wait_ge(s_vec, n - NS + 1)
                s.wait_ge(s_din[r], 32 * (n // NS + 1))
                d3 = dtile[r].ap().rearrange("p (t k) -> p t k", k=NBITS)
                A, Bw = wkA[r], wkB[r]
                s.copy(spans(A), d3)
                # out-copy of tile n-2 here: its ~1.9us also spaces the
                # dependent in-copy -> stage-0-copy pair
                if n >= 2:
                    m = n - 2
                    o = m % NS
                    s.wait_ge(s_vec, m + 1)   # B_m holds tile m's result
                    if m >= NS:
                        # otile slot o drained for tile m-NS
                        s.wait_ge(s_dout[o], 16 * (m // NS))
                    o3 = otile[o].ap().rearrange("p (t k) -> p t k", k=NBITS)
                    s.copy(o3, spans(wkB[o])).then_inc(s_oc, 1)
                elif sim_sync:
                    s.drain()
                if sim_sync:
                    s.drain()
                # stage 0 shifted copy: B = A >> 1 lane (src reads one guard
                # zero at the span head)
                s.copy(spans(Bw), spans(A, GUARD - 1)).then_inc(s_pre, 1)
            for m in (nt - 2, nt - 1):
                o = m % NS
                s.wait_ge(s_vec, m + 1)
                s.wait_ge(s_dout[o], 16 * (m // NS))
                o3 = otile[o].ap().rearrange("p (t k) -> p t k", k=NBITS)
                s.copy(o3, spans(wkB[o])).then_inc(s_oc, 1)

        @block.vector
        def _(v):
            # zero the work tiles once so every guard lane reads 0
            for j in range(NS):
                v.memset(wkA[j].ap(), 0.0)
            for j in range(NS):
                v.memset(wkB[j].ap(), 0.0)
            if sim_sync:
                v.drain()
            # re-clear a sliver as the inc carrier: an ENGINE op (not a
            # seq-only sem_inc) so the zeroing is provably ordered before it
            v.memset(wkB[0].ap()[:, 0:2], 0.0).then_inc(s_zero, 1)
            for n in range(nt):
                r = n % NS
                A, Bw = wkA[r], wkB[r]
                # copy_predicated requires an integer-typed mask; the f32 bit
                # pattern of 1.0 is nonzero, so a bitcast view keeps semantics
                st3 = stile[r].ap().rearrange("p (t j) -> p t j", j=NCTRL)
                sti3 = stile[r].ap().bitcast(i32).rearrange(
                    "p (t j) -> p t j", j=NCTRL
                )
                m3 = msk.ap().rearrange("p (t k) -> p t k", k=PITCH32)[
                    :, :, 0:W32
                ]

                def bitbrd(i):
                    # stage-i select bit (shift[:, 5-i]) broadcast across the
                    # 32 int32 pair-lanes of its span
                    return sti3[:, :, 5 - i:6 - i].broadcast_to(
                        [P, SPANS, W32]
                    )

                # stage-0 inverted mask (bit - 1: nonzero iff bit clear),
                # issued BEFORE the s_pre wait — it only needs the DMA'd
                # stile, so it hides inside ScalarE's chain
                v.wait_ge(s_din[r], 32 * (n // NS + 1))
                v.tensor_scalar(
                    m3,
                    st3[:, :, 5:6].broadcast_to([P, SPANS, W32]),
                    1.0,
                    None,
                    Alu.subtract,
                )
                v.wait_ge(s_pre, n + 1)
                if sim_sync:
                    v.drain()
                # stage 0: B holds shift-by-1(A); predicate the unshifted A
                # over it where the select bit is CLEAR.
                v.copy_predicated(spans32(Bw), m3, spans32(A))
                # stages 1..5 in place on B, reversed inner order
                for i in range(1, 6):
                    w = (1 << i) // 2
                    if sim_sync:
                        v.drain()
                    ins = v.copy_predicated(
                        spans32(Bw, rev=True),
                        bitbrd(i),
                        spans32(Bw, GUARD32 - w, rev=True),
                    )
                    if i == 5:
                        ins.then_inc(s_vec, 1)

    return nc


def _get(rows):
    if rows not in _built:
        _built[rows] = build(rows)
    return _built[rows]


def run_cores(data, shift, rows, trace=False):
    from concourse.bass_utils import run_bass_kernel_spmd

    nc = _get(rows)
    ncores = data.shape[0] // rows
    in_maps = [
        {
            "data": np.ascontiguousarray(data[i * rows:(i + 1) * rows]),
            "shift": np.ascontiguousarray(shift[i * rows:(i + 1) * rows]),
        }
        for i in range(ncores)
    ]
    res = run_bass_kernel_spmd(nc, in_maps, list(range(ncores)), trace=trace)
    full = np.concatenate([res.results[i]["out"] for i in range(ncores)], axis=0)
    return full, res


def kernel(data, shift):
    data = np.ascontiguousarray(np.asarray(data), dtype=np.float32)
    shift = np.ascontiguousarray(np.asarray(shift), dtype=np.float32)
    full, _ = run_cores(data, shift, R_FULL)
    return full.astype(np.float32, copy=False)


# revision 39
# speedup vs baseline: 1.7816x; 1.2701x over previous
"""Barrel shifter right 64 (zero-fill), batch 2097152, on 8 NeuronCores.

Layout: row-major. Each SBUF work tile holds 4096 rows: partition p carries 32
consecutive rows (spans), each span padded to 96 bf16 elements (32 zero guard +
64 data). A stage's shift-by-sa is a free-dim offset view whose low lanes read
the zero guard.

Engine split — DVE (the only engine with copy_predicated, which has no fast
perf mode) runs NOTHING but the six predicated mux copies:
  - DVE: per stage one 1x copy_predicated on int32 BF16 PAIRS (the mux select
    is per row, so adjacent lanes share it; every shift >= 2 is pair-aligned),
    halving the 1x element count. Stages 1..5 predicate directly on the raw
    f32 select bit broadcast across the span, bitcast to int32 (nonzero test;
    1.0f != 0) — no mask materialization. They run in place with REVERSED
    innermost APs: processing high->low guarantees each shifted read (at k-w)
    precedes that position's overwrite, for any w.
  - Stage 0 (sa=1) has an odd bf16 offset that breaks the int32 pairing, so
    it takes the copy + inverted-predicate form: ScalarE writes the shifted
    copy A->B and DVE predicates the unshifted A over it where the select
    bit is CLEAR. The inverted mask (select bit minus 1: nonzero exactly
    when clear) is the one materialized mask; DVE computes it before its
    s_pre wait so it hides inside ScalarE's chain.
  - ScalarE: f32->bf16 in-copy, the bf16->f32 out-copy of tile n-2 (which
    also spaces the dependent in-copy -> stage-0-copy pair), and stage-0's
    shift-by-1 copy. GPSIMD is deliberately idle: its slow software tensor
    ops contend with DVE for SBUF ports (measured: a 9us/tile GPSIMD copy
    doubled every DVE predicated-copy's duration).

All tile classes rotate through 3 slots and the output DMA lags 3 tiles, so
the sync engine issues input DMAs back-to-back and each tile's ~7us HBM load
is in flight ~2 tiles before its consumption.
"""

import sys

if "/opt/trn_rl_repo" not in sys.path:
    sys.path.insert(0, "/opt/trn_rl_repo")

import numpy as np

B_TOTAL = 2097152
NBITS = 64
NCTRL = 6
NCORES = 8
R_FULL = B_TOTAL // NCORES  # 262144 rows per core

P = 128
SPANS = 32                  # rows per partition per tile
TILE_ROWS = P * SPANS       # 4096
PITCH = 96                  # bf16 elems: guard(32) + bits(64)
GUARD = 32
W32 = NBITS // 2            # 32 int32 lanes per span
PITCH32 = PITCH // 2        # 48
GUARD32 = GUARD // 2        # 16
FD = SPANS * NBITS          # 2048
SFD = SPANS * NCTRL         # 192
NS = 3                      # slot count for every rotating tile class

_built = {}


def build(rows, sim_sync=False):
    # sim_sync inserts drains between same-engine dependent ops purely to
    # satisfy CoreSim's conservative OOO-engine race model; on hardware the
    # ops are all far above the ~266ns write-flush threshold (the proven
    # baseline relies on the same adjacency).
    import concourse.bass as bass
    from concourse import mybir

    f32 = mybir.dt.float32
    bf16 = mybir.dt.bfloat16
    i32 = mybir.dt.int32
    Alu = mybir.AluOpType
    Act = mybir.ActivationFunctionType

    nt = rows // TILE_ROWS
    assert rows % TILE_ROWS == 0
    assert nt >= 2 * NS

    nc = bass.Bass()
    data = nc.declare_dram_parameter("data", [rows, NBITS], f32, isOutput=False)
    shift = nc.declare_dram_parameter("shift", [rows, NCTRL], f32, isOutput=False)
    out = nc.declare_dram_parameter("out", [rows, NBITS], f32, isOutput=True)

    data_r = data.rearrange("(n p t) k -> n p (t k)", p=P, t=SPANS)
    shift_r = shift.rearrange("(n p t) k -> n p (t k)", p=P, t=SPANS)
    out_r = out.rearrange("(n p t) k -> n p (t k)", p=P, t=SPANS)

    def sb(name, shape, dt):
        return nc.alloc_sbuf_tensor(name, shape, dt)

    dtile = [sb(f"dtile{j}", [P, FD], f32) for j in range(NS)]
    stile = [sb(f"stile{j}", [P, SFD], f32) for j in range(NS)]
    # stage-0 masks share the 48-pitch span structure of the work tiles so
    # their APs lower with the same dimension structure as copy_predicated's
    msk = [sb(f"msk{j}", [P, SPANS * PITCH32], i32) for j in range(NS)]
    wkA = [sb(f"wkA{j}", [P, SPANS * PITCH], bf16) for j in range(NS)]
    wkB = [sb(f"wkB{j}", [P, SPANS * PITCH], bf16) for j in range(NS)]
    otile = [sb(f"otile{j}", [P, FD], f32) for j in range(NS)]

    def spans(t, off=GUARD):
        # [P, SPANS, NBITS] bf16 view at span-local offset `off`
        return t.ap().rearrange("p (t c) -> p t c", c=PITCH)[:, :, off:off + NBITS]

    def spans32(t, off=GUARD32, rev=False):
        # [P, SPANS, W32] int32 (bf16-pair) view at span-local int32 offset
        v = t.ap().bitcast(i32).rearrange("p (t c) -> p t c", c=PITCH32)[
            :, :, off:off + W32
        ]
        return v[:, :, ::-1] if rev else v

    with (
        nc.Block() as block,
        nc.semaphore("s_din0") as s_din0,
        nc.semaphore("s_din1") as s_din1,
        nc.semaphore("s_din2") as s_din2,
        nc.semaphore("s_dout0") as s_dout0,
        nc.semaphore("s_dout1") as s_dout1,
        nc.semaphore("s_dout2") as s_dout2,
        nc.semaphore("s_pre") as s_pre,
        nc.semaphore("s_vec") as s_vec,
        nc.semaphore("s_oc") as s_oc,
        nc.semaphore("s_zero") as s_zero,
    ):
        s_din = [s_din0, s_din1, s_din2]
        s_dout = [s_dout0, s_dout1, s_dout2]

        @block.sync
        def _(sp):
            for n in range(nt):
                r = n % NS
                if n >= NS:
                    # input slot r free once ScalarE's in-copy (dtile) and
                    # DVE (stile, read by the predicates) of tile n-NS done
                    sp.wait_ge(s_pre, n - NS + 1)
                    sp.wait_ge(s_vec, n - NS + 1)
                sp.dma_start(
                    out=dtile[r].ap(), in_=data_r[n]
                ).then_inc(s_din[r], 16)
                sp.dma_start(
                    out=stile[r].ap(), in_=shift_r[n]
                ).then_inc(s_din[r], 16)
                if n >= NS:
                    m = n - NS
                    sp.wait_ge(s_oc, m + 1)  # out-copy of tile m done
                    sp.dma_start(
                        out=out_r[m], in_=otile[m % NS].ap()
                    ).then_inc(s_dout[m % NS], 16)
            for m in range(nt - NS, nt):
                sp.wait_ge(s_oc, m + 1)
                sp.dma_start(
                    out=out_r[m], in_=otile[m % NS].ap()
                ).then_inc(s_dout[m % NS], 16)
            for j in range(NS):
                sp.wait_ge(s_dout[j], 16 * len(range(j, nt, NS)))

        @block.scalar
        def _(s):
            s.wait_ge(s_zero, 1)
            for n in range(nt):
                r = n % NS
                if n >= NS:
                    # work pair r free once DVE(n-NS) is done with it (its B
                    # was read out by this engine's own out-copy, in order)
                    s.wait_ge(s_vec, n - NS + 1)
                s.wait_ge(s_din[r], 32 * (n // NS + 1))
                d3 = dtile[r].ap().rearrange("p (t k) -> p t k", k=NBITS)
                A, Bw = wkA[r], wkB[r]
                s.copy(spans(A), d3)
                # out-copy of tile n-2 here: its ~1.9us also spaces the
                # dependent in-copy -> stage-0-copy pair
                if n >= 2:
                    m = n - 2
                    o = m % NS
                    s.wait_ge(s_vec, m + 1)   # B_m holds tile m's result
                    if m >= NS:
                        # otile slot o drained for tile m-NS
                        s.wait_ge(s_dout[o], 16 * (m // NS))
                    o3 = otile[o].ap().rearrange("p (t k) -> p t k", k=NBITS)
                    s.copy(o3, spans(wkB[o])).then_inc(s_oc, 1)
                elif sim_sync:
                    s.drain()
                if sim_sync:
                    s.drain()
                # stage 0 shifted copy: B = A >> 1 lane (src reads one guard
                # zero at the span head)
                s.copy(spans(Bw), spans(A, GUARD - 1)).then_inc(s_pre, 1)
            for m in (nt - 2, nt - 1):
                o = m % NS
                s.wait_ge(s_vec, m + 1)
                s.wait_ge(s_dout[o], 16 * (m // NS))
                o3 = otile[o].ap().rearrange("p (t k) -> p t k", k=NBITS)
                s.copy(o3, spans(wkB[o])).then_inc(s_oc, 1)

        @block.vector
        def _(v):
            # zero only the guard lanes that are ever read: B's full 32-elem
            # span guards (the shifted predicate sources reach down to span
            # offset 0) and A's single element at GUARD-1 (read by the
            # stage-0 shifted copy). Data regions are written before read.
            for j in range(NS):
                ga = wkA[j].ap().rearrange("p (t c) -> p t c", c=PITCH)[
                    :, :, GUARD - 1:GUARD
                ]
                v.memset(ga, 0.0)
            for j in range(NS):
                gb = wkB[j].ap().rearrange("p (t c) -> p t c", c=PITCH)[
                    :, :, 0:GUARD
                ]
                v.memset(gb, 0.0)
            if sim_sync:
                v.drain()
            # re-clear a sliver as the inc carrier: an ENGINE op (not a
            # seq-only sem_inc) so the zeroing is provably ordered before it
            v.memset(wkB[0].ap()[:, 0:2], 0.0).then_inc(s_zero, 1)
            for n in range(nt):
                r = n % NS
                A, Bw = wkA[r], wkB[r]
                # copy_predicated requires an integer-typed mask; the f32 bit
                # pattern of 1.0 is nonzero, so a bitcast view keeps semantics
                st3 = stile[r].ap().rearrange("p (t j) -> p t j", j=NCTRL)
                sti3 = stile[r].ap().bitcast(i32).rearrange(
                    "p (t j) -> p t j", j=NCTRL
                )
                m3 = msk[r].ap().rearrange("p (t k) -> p t k", k=PITCH32)[
                    :, :, 0:W32
                ]

                def bitbrd(i):
                    # stage-i select bit (shift[:, 5-i]) broadcast across the
                    # 32 int32 pair-lanes of its span
                    return sti3[:, :, 5 - i:6 - i].broadcast_to(
                        [P, SPANS, W32]
                    )

                # stage-0 inverted mask (bit - 1: nonzero iff bit clear),
                # issued BEFORE the s_pre wait — it only needs the DMA'd
                # stile, so it hides inside ScalarE's chain
                v.wait_ge(s_din[r], 32 * (n // NS + 1))
                v.tensor_scalar(
                    m3,
                    st3[:, :, 5:6].broadcast_to([P, SPANS, W32]),
                    1.0,
                    None,
                    Alu.subtract,
                )
                v.wait_ge(s_pre, n + 1)
                if sim_sync:
                    v.drain()
                # stage 0: B holds shift-by-1(A); predicate the unshifted A
                # over it where the select bit is CLEAR.
                v.copy_predicated(spans32(Bw), m3, spans32(A))
                # stages 1..5 in place on B, reversed inner order
                for i in range(1, 6):
                    w = (1 << i) // 2
                    if sim_sync:
                        v.drain()
                    ins = v.copy_predicated(
                        spans32(Bw, rev=True),
                        bitbrd(i),
                        spans32(Bw, GUARD32 - w, rev=True),
                    )
                    if i == 5:
                        ins.then_inc(s_vec, 1)

    return nc


def _get(rows):
    if rows not in _built:
        _built[rows] = build(rows)
    return _built[rows]


def run_cores(data, shift, rows, trace=False):
    from concourse.bass_utils import run_bass_kernel_spmd

    nc = _get(rows)
    ncores = data.shape[0] // rows
    in_maps = [
        {
            "data": np.ascontiguousarray(data[i * rows:(i + 1) * rows]),
            "shift": np.ascontiguousarray(shift[i * rows:(i + 1) * rows]),
        }
        for i in range(ncores)
    ]
    res = run_bass_kernel_spmd(nc, in_maps, list(range(ncores)), trace=trace)
    full = np.concatenate([res.results[i]["out"] for i in range(ncores)], axis=0)
    return full, res


def kernel(data, shift):
    data = np.ascontiguousarray(np.asarray(data), dtype=np.float32)
    shift = np.ascontiguousarray(np.asarray(shift), dtype=np.float32)
    full, _ = run_cores(data, shift, R_FULL)
    return full.astype(np.float32, copy=False)
